# revision 14
# baseline (speedup 1.0000x reference)
"""Distributed exact-kNN kernel for Trainium2 (8 NeuronCores).

Problem: B=2048 queries (512-d), N=100000 fitted rows, k=5 nearest
neighbors by squared L2; output = mean of the 5 neighbor vectors.

Strategy (shard X_fit along N, 12500 rows/core; fold-in-PSUM epilogue):
  - Host quantizes to INTEGER-valued fp8: qd = fp8(rint(16 q)),
    xd = fp8(rint(8 x)).  The device matmul accumulates an EXACT
    integer in f32 PSUM: dot = qd.xd = 128*(q.x) + quant-noise
    = 64*(2 q.x) + noise.
  - A K=1 fp16 "ones" matmul adds r_c = fp16(64*xs_c), xs_c =
    512 - ||x_c||^2 (fp16 => integer, |err| <= 8 = 0.125 units), so
    PSUM holds the full integer score m = 64*(2 q.x + xs) + eps,
    |m| << 2^16.
  - The epilogue folds each 2560-col piece down to 256 "class slots"
    (col mod 256) with pairwise tensor_tensor(max) ops.  ACT
    Copy-drains PSUM to int16 SBUF (scores fit int16 exactly); DVE
    folds int16 pairs at 2x rate (2x_1p), plus one mixed
    PSUM-f32-vs-int16 fold for the odd 512-col tail (GpSimd cannot
    run TensorTensor on this ISA; it is unused).
  - DVE packs the class tag exactly: w = m*256 + iota (f32 exact,
    |w| < 2^24), then max8 over 256 slots -> top-8 packed values per
    (query, piece).
  - Host: merge 8*40 candidates/query, top-16 by value, expand the
    mod-256 class ambiguity (10 rows each), dedup, exact f32 re-rank,
    top-k, mean.  Exact re-rank makes all quantization noise
    irrelevant as long as the true top-5 survive in the candidate
    set; fold collisions are harmless because a slot's class IS its
    ambiguity set (validated in simulation: 0/2048 rows mismatch).
"""

import sys

if "/opt/trn_rl_repo" not in sys.path:
    sys.path.insert(0, "/opt/trn_rl_repo")

import numpy as np
import ml_dtypes

# ---- problem geometry (hardcoded per spec) ----
B = 2048  # queries
D = 512  # feature dim
N = 100000  # fitted rows
NCORES = 8
NSHARD = N // NCORES  # 12500
NPAD = 12800  # padded shard length
PIECE = 2560  # piece processed per inner iteration
QB = 128  # queries per block
NQB = B // QB  # 16
DCH = D // 128  # 4 contraction chunks
NPIECES = NPAD // PIECE  # 5

AQ = 16.0  # query pre-scale before integer rounding
AX = 8.0  # fitted-row pre-scale
RSCALE = 64.0  # r = fp16(RSCALE * xs); psum dot = 64*(2 q.x)
R_PAD = -20000.0  # fp16-exact pad sentinel (never wins a fold)
IOTA_MOD = 256  # class modulus
NSLOT = 256  # final slots per piece
W_VALID_MIN = -4_800_000  # host filter (in *256 integer units)

REPEAT = 1  # dev: run the whole pipeline N times (for overhead-cancelling timing)
SPOOL_BUFS = 6
UPOOL_BUFS = 6
WPOOL_BUFS = 8

_compiled = None


def _tsp(eng, nc, mybir, out, in0, imm, in1):
    """out = (in0 * imm) + in1 — all-arith TensorScalarPtr (DVE)."""
    return eng.add_instruction(
        mybir.InstTensorScalarPtr(
            name=nc.get_next_instruction_name(),
            is_scalar_tensor_tensor=True,
            op0=mybir.AluOpType.mult,
            op1=mybir.AluOpType.add,
            ins=[
                eng.lower_ap(in0),
                mybir.ImmediateValue(dtype=mybir.dt.float32, value=imm),
                eng.lower_ap(in1),
            ],
            outs=[eng.lower_ap(out)],
        )
    )


def _build():
    import concourse.mybir as mybir
    import concourse.tile as tile
    from concourse import bacc

    nc = bacc.Bacc(None, target_bir_lowering=False)

    fp8 = mybir.dt.float8e4
    fp16 = mybir.dt.float16
    f32 = mybir.dt.float32
    i16 = mybir.dt.int16
    qT = nc.dram_tensor("qT", [DCH, 128, B], fp8, kind="ExternalInput")
    xT = nc.dram_tensor("xT", [DCH, 128, NPAD], fp8, kind="ExternalInput")
    r = nc.dram_tensor("r", [1, NPAD], fp16, kind="ExternalInput")
    ones = nc.dram_tensor("ones", [1, 128], fp16, kind="ExternalInput")
    iotaf = nc.dram_tensor("iotaf", [QB, NSLOT], f32, kind="ExternalInput")
    cand = nc.dram_tensor("cand", [NQB, QB, NPIECES, 8], f32, kind="ExternalOutput")

    Copy = mybir.ActivationFunctionType.Copy
    Max = mybir.AluOpType.max
    DR = mybir.MatmulPerfMode.DoubleRow

    with tile.TileContext(nc) as tc:
        with (
            tc.tile_pool(name="persist", bufs=1) as pp,
            tc.tile_pool(name="xpool", bufs=2) as xp_pool,
            tc.tile_pool(name="spool", bufs=SPOOL_BUFS) as sp,
            tc.tile_pool(name="upool", bufs=UPOOL_BUFS) as up,
            tc.tile_pool(name="wpool", bufs=WPOOL_BUFS) as wp,
            tc.tile_pool(name="accp", bufs=2) as ap_pool,
            tc.tile_pool(name="psAB", bufs=3, space="PSUM") as psab,
            tc.tile_pool(name="psC", bufs=2, space="PSUM") as psc,
        ):
            qT_t = pp.tile([128, DCH, B], fp8, name="qTt")
            nc.sync.dma_start(qT_t[:], qT[:].rearrange("c p b -> p c b"))
            r_t = pp.tile([1, NPAD], fp16, name="r_t")
            nc.sync.dma_start(r_t[:], r[:])
            ones_t = pp.tile([1, 128], fp16, name="ones_t")
            nc.sync.dma_start(ones_t[:], ones[:])
            iota_t = pp.tile([QB, NSLOT], f32, name="iota_t")
            nc.sync.dma_start(iota_t[:], iotaf[:])

            # Epilogue chain per iteration: ACT drains -> DVE folds ->
            # DVE pack+max8.  The post-drain chain is DVE-only, so the
            # in-order DVE queue pipelines naturally across iterations.
            for rep in range(REPEAT):
                for p in range(NPIECES):
                    lo = p * PIECE
                    xp = xp_pool.tile([128, DCH, PIECE], fp8, tag="xp", name="xp")
                    nc.sync.dma_start(
                        xp[:], xT[:, :, lo : lo + PIECE].rearrange("c p n -> p c n")
                    )
                    acc = ap_pool.tile([QB, NQB, 8], f32, tag="acc", name="acc")

                    for qb in range(NQB):
                        qcols = slice(qb * QB, (qb + 1) * QB)
                        pA = psab.tile([QB, 1024], f32, tag="ps", name="psA")
                        pB = psab.tile([QB, 1024], f32, tag="ps", name="psB")
                        pC = psc.tile([QB, 512], f32, tag="psc", name="psC")
                        # (tile, col offset in tile, piece col)
                        chunks = [
                            (pA, 0, 0), (pA, 512, 512),
                            (pB, 0, 1024), (pB, 512, 1536),
                            (pC, 0, 2048),
                        ]
                        for t, sub, off in chunks:
                            nc.tensor.matmul(
                                t[:, sub : sub + 512],
                                ones_t[:],
                                r_t[:, lo + off : lo + off + 512],
                                start=True,
                                stop=False,
                            )
                        for kk in range(0, DCH, 2):
                            for t, sub, off in chunks:
                                nc.tensor.matmul(
                                    t[:, sub : sub + 512],
                                    qT_t[:, kk : kk + 2, qcols],
                                    xp[:, kk : kk + 2, off : off + 512],
                                    start=False,
                                    stop=(kk + 2 >= DCH),
                                    perf_mode=DR,
                                )
                        # ACT drains A, B fully and half of C to int16
                        sA = sp.tile([QB, 1024], i16, tag="sA", name="sA")
                        sB = sp.tile([QB, 1024], i16, tag="sB", name="sB")
                        sC = sp.tile([QB, 256], i16, tag="sC", name="sC")
                        nc.scalar.activation(out=sA[:], in_=pA[:, 0:1024], func=Copy)
                        nc.scalar.activation(out=sB[:], in_=pB[:, 0:1024], func=Copy)
                        nc.scalar.activation(out=sC[:], in_=pC[:, 256:512], func=Copy)
                        # DVE fold tree (int16 at 2x; one mixed fold for C)
                        fA = up.tile([QB, 512], i16, tag="fA", name="fA")
                        nc.vector.tensor_tensor(fA[:], sA[:, 0:512], sA[:, 512:1024], Max)
                        fB = up.tile([QB, 512], i16, tag="fB", name="fB")
                        nc.vector.tensor_tensor(fB[:], sB[:, 0:512], sB[:, 512:1024], Max)
                        fC = up.tile([QB, 256], i16, tag="fC", name="fC")
                        nc.vector.tensor_tensor(fC[:], pC[:, 0:256], sC[:], Max)
                        L2 = wp.tile([QB, 512], i16, tag="L2", name="L2")
                        nc.vector.tensor_tensor(L2[:], fA[:], fB[:], Max)
                        L3 = wp.tile([QB, 256], i16, tag="L3", name="L3")
                        nc.vector.tensor_tensor(L3[:], L2[:, 0:256], L2[:, 256:512], Max)
                        L4 = wp.tile([QB, 256], i16, tag="L4", name="L4")
                        nc.vector.tensor_tensor(L4[:], L3[:], fC[:], Max)
                        # pack class tag and top-8
                        pk = wp.tile([QB, 256], f32, tag="pk", name="pk")
                        _tsp(nc.vector, nc, mybir, pk[:], L4[:], 256.0, iota_t[:])
                        nc.vector.max(out=acc[:, qb, :], in_=pk[:])

                    nc.sync.dma_start(
                        cand[:, :, p, :].rearrange("q r j -> r q j"), acc[:]
                    )
    nc.compile()
    return nc


def _get_compiled():
    global _compiled
    if _compiled is None:
        _compiled = _build()
    return _compiled


def _prepare_inputs(q, X):
    """Build per-core in_maps. q: [B, D] f32, X: [N, D] f32."""
    from concourse import mybir

    fp8 = mybir.dt.np(mybir.dt.float8e4)
    fp16 = np.float16
    qd = np.rint(AQ * q).astype(fp8)  # [B, D] integer-valued fp8
    qT_np = np.ascontiguousarray(qd.T.reshape(DCH, 128, B))
    ones_np = np.ones((1, 128), dtype=fp16)
    iota_np = np.ascontiguousarray(
        np.broadcast_to(
            np.arange(NSLOT, dtype=np.float32)[None, :], (QB, NSLOT)
        )
    )

    in_maps = []
    for core in range(NCORES):
        Xi = X[core * NSHARD : (core + 1) * NSHARD]
        xd = np.rint(AX * Xi).astype(fp8)  # [NSHARD, D]
        xT_np = np.zeros((DCH, 128, NPAD), dtype=fp8)
        xT_np[:, :, :NSHARD] = xd.T.reshape(DCH, 128, NSHARD)
        xsq = np.einsum("nd,nd->n", Xi, Xi, dtype=np.float32)
        rrow = np.full((1, NPAD), R_PAD, dtype=fp16)
        rrow[0, :NSHARD] = np.rint(RSCALE * (512.0 - xsq)).astype(fp16)
        in_maps.append(
            {"qT": qT_np, "xT": xT_np, "r": rrow, "ones": ones_np, "iotaf": iota_np}
        )
    return in_maps


def _run_device(in_maps, trace=False, tmpdir=None):
    from concourse.bass_utils import run_bass_kernel_spmd

    nc = _get_compiled()
    kwargs = {}
    if trace:
        kwargs = {"trace": True, "tmpdir": tmpdir}
    return run_bass_kernel_spmd(nc, in_maps, core_ids=list(range(NCORES)), **kwargs)


def _merge_host(cand_all, q, X, k):
    """cand_all: [NCORES, NQB, QB, NPIECES, 8] f32. Returns [B, 1, D] f32."""
    w = np.rint(cand_all.astype(np.float64)).astype(np.int64)
    w = w.reshape(NCORES, B, NPIECES * 8)
    w_b = np.moveaxis(w, 0, 1).reshape(B, NCORES * NPIECES * 8)  # [B, 320]
    ncand = NPIECES * 8
    cores = np.repeat(np.arange(NCORES), ncand)[None, :]
    pieces = np.tile(np.repeat(np.arange(NPIECES), 8), NCORES)[None, :]

    C = max(16, 3 * k)
    top = np.argpartition(-w_b, C, axis=1)[:, :C]  # [B, C]
    wt = np.take_along_axis(w_b, top, axis=1)
    core_t = np.take_along_axis(np.broadcast_to(cores, w_b.shape), top, axis=1)
    piece_t = np.take_along_axis(np.broadcast_to(pieces, w_b.shape), top, axis=1)
    iota_t = wt % IOTA_MOD  # [B, C] in [0, 256)
    namb = PIECE // IOTA_MOD  # 10
    offs = np.arange(namb) * IOTA_MOD
    local = piece_t[:, :, None] * PIECE + iota_t[:, :, None] + offs[None, None, :]
    rows = core_t[:, :, None] * NSHARD + local  # [B, C, 10]
    valid = (local < NSHARD) & (wt[:, :, None] > W_VALID_MIN)
    rows = np.where(valid, rows, 0).reshape(B, C * namb)
    valid = valid.reshape(B, C * namb)

    # dedup repeated rows per query (ambiguity sets can overlap)
    order = np.argsort(rows, axis=1)
    rs = np.take_along_axis(rows, order, axis=1)
    dup_s = np.zeros_like(rs, dtype=bool)
    dup_s[:, 1:] = rs[:, 1:] == rs[:, :-1]
    dup = np.zeros_like(dup_s)
    np.put_along_axis(dup, order, dup_s, axis=1)

    out = np.empty((B, D), dtype=np.float32)
    CH = 512
    for i in range(0, B, CH):
        rr = rows[i : i + CH]
        Xg = X[rr]  # [CH, C*10, D]
        xsq_g = np.einsum("bcd,bcd->bc", Xg, Xg, dtype=np.float32)
        d2 = xsq_g - 2.0 * np.einsum("bcd,bd->bc", Xg, q[i : i + CH], dtype=np.float32)
        d2 = np.where(valid[i : i + CH] & ~dup[i : i + CH], d2, np.inf)
        win = np.argpartition(d2, k - 1, axis=1)[:, :k]
        neigh = np.take_along_axis(Xg, win[:, :, None], axis=1)
        out[i : i + CH] = neigh.mean(axis=1, dtype=np.float32)
    return out.reshape(B, 1, D).astype(np.float32)


def kernel(x_enc, X_fit, n_neighbors, _trace=False, _tmpdir=None):
    q = np.asarray(x_enc, dtype=np.float32).reshape(B, D)
    X = np.asarray(X_fit, dtype=np.float32)
    k = int(n_neighbors)
    in_maps = _prepare_inputs(q, X)
    res = _run_device(in_maps, trace=_trace, tmpdir=_tmpdir)
    cand_all = np.stack([res.results[c]["cand"] for c in range(NCORES)])
    out = _merge_host(cand_all, q, X, k)
    if _trace:
        return out, res
    return out
